# revision 9
# baseline (speedup 1.0000x reference)
"""Trainium2 Bass kernel for DetectPeaks (sliding-window NMS + top-2).

Computes, for xcorr [32, 3, 64, 8192] f32:
    x = |xcorr|
    smax = sliding max over time, window 301 (centered, clipped)
    scores = where(smax == x, x, 0)
    top2 values + indices along time  -> ([32,3,64,2] f32, [32,3,64,2] int32)

Strategy: flatten to 6144 independent rows, shard 768 rows per core across
8 cores (data parallel, no communication).  Per 128-row tile:
  - abs on the scalar engine (in place, in a -1.0-padded buffer)
  - van Herk / Gil-Werman sliding max: per-301-block prefix/suffix max scans
    (tensor_tensor_scan with op=max on DVE), then
    smax[t] = max(S[t], P[t+300])
  - scores' = x + 1e30*(x - smax): exactly x at peaks (x==smax), very
    negative otherwise, so top-k of scores' == top-k of the reference's
    masked scores (for rows with >= 2 peaks; random data has ~27 peaks/row)
  - top-8 values + indices per row via DVE max / max_index, keep 2
"""

import numpy as np

NB, NC, NX, NT = 32, 3, 64, 8192
KERNEL = 301
PAD = KERNEL // 2  # 150
B = KERNEL  # van Herk block size
NBLK = 29  # ceil((PAD + NT + PAD) / B) -> cover xp coords [0, 8491]
LPAD = NBLK * B  # 8729
N_CORES = 8
ROWS = NB * NC * NX  # 6144
ROWS_PER_CORE = ROWS // N_CORES  # 768
P_DIM = 128
NTILE = ROWS_PER_CORE // P_DIM  # 6
BIG = 1.0e30

_cached = None


def _build(rows_per_core=ROWS_PER_CORE):
    import concourse.mybir as mybir
    from concourse.bacc import Bacc
    from concourse.tile import TileContext

    f32 = mybir.dt.float32
    Alu = mybir.AluOpType
    n_tiles = rows_per_core // P_DIM

    # Bacc (not plain Bass): its finalize() runs generate_event_semaphores,
    # which splits multi-sem waits into EventSemaphore prefixes — TRN2
    # instructions only have a single wait slot.
    nc = Bacc(None, target_bir_lowering=False)
    x_in = nc.dram_tensor("x", [rows_per_core, NT], f32, kind="ExternalInput")
    out_vals = nc.dram_tensor("out_vals", [rows_per_core, 8], f32, kind="ExternalOutput")
    out_idx = nc.dram_tensor("out_idx", [rows_per_core, 8], mybir.dt.uint32, kind="ExternalOutput")

    with TileContext(nc) as tc:
        with (
            tc.tile_pool(name="big", bufs=2) as bigpool,
            tc.tile_pool(name="scan", bufs=1) as scanpool,
            tc.tile_pool(name="sc", bufs=2) as scpool,
            tc.tile_pool(name="small", bufs=2) as smallpool,
        ):
            for i in range(n_tiles):
                rows = slice(i * P_DIM, (i + 1) * P_DIM)
                xp = bigpool.tile([P_DIM, LPAD], f32, tag="xp")
                interior = xp[:, PAD:PAD + NT]
                nc.sync.dma_start(interior, x_in[rows, :])
                # Pads + abs all on the scalar engine: the pad memzeros absorb
                # the WAR-vs-DVE wait, abs then only waits on the input DMA, and
                # the scans below wait only on ACT.  This keeps every ACT / DVE
                # scan instruction at <=1 sync wait (the ACT and scan ISA
                # structs have very few wait-command slots).
                nc.scalar.memzero(xp[:, 0:PAD])
                nc.scalar.memzero(xp[:, PAD + NT:LPAD])
                nc.scalar.activation(interior, interior, mybir.ActivationFunctionType.Abs)

                S = scanpool.tile([P_DIM, LPAD], f32, tag="S")
                Pm = scanpool.tile([P_DIM, LPAD], f32, tag="P")
                for j in range(NBLK):
                    lo, hi = j * B, (j + 1) * B
                    fwd_in = xp[:, lo:hi]
                    nc.vector.tensor_tensor_scan(
                        Pm[:, lo:hi], fwd_in, fwd_in, -1.0, op0=Alu.max, op1=Alu.max
                    )
                for j in range(NBLK):
                    lo, hi = j * B, (j + 1) * B
                    rev = slice(hi - 1, lo - 1 if lo > 0 else None, -1)
                    rev_in = xp[:, rev]
                    nc.vector.tensor_tensor_scan(
                        S[:, rev], rev_in, rev_in, -1.0, op0=Alu.max, op1=Alu.max
                    )

                # smax[t] = max(S[t], P[t+300]); t in padded coords = x coords
                m = scpool.tile([P_DIM, NT], f32, tag="m")
                nc.vector.tensor_tensor(
                    out=m, in0=S[:, 0:NT], in1=Pm[:, 2 * PAD:2 * PAD + NT], op=Alu.max
                )
                # m <- (x >= smax)  == peak mask (1.0 / 0.0)
                nc.vector.tensor_tensor(out=m, in0=interior, in1=m, op=Alu.is_ge)
                # m <- mask * x   (exactly x at peaks, 0 elsewhere == reference scores)
                nc.vector.tensor_tensor(out=m, in0=m, in1=interior, op=Alu.mult)

                v8 = smallpool.tile([P_DIM, 8], f32, tag="v8")
                i8 = smallpool.tile([P_DIM, 8], mybir.dt.uint32, tag="i8")
                nc.vector.max(out=v8, in_=m)
                nc.vector.max_index(out=i8, in_max=v8, in_values=m)
                nc.sync.dma_start(out_vals[rows, :], v8)
                nc.sync.dma_start(out_idx[rows, :], i8)
    return nc


def _get_module():
    global _cached
    if _cached is None:
        _cached = _build()
        # run_bass_via_pjrt serializes the module as-is; Bacc.finalize()
        # runs register allocation + event-semaphore legalization.
        _cached.finalize()
    return _cached


def run(xcorr: np.ndarray, trace: bool = False, **spmd_kwargs):
    from concourse.bass_utils import run_bass_kernel_spmd

    x = np.ascontiguousarray(np.asarray(xcorr, dtype=np.float32).reshape(ROWS, NT))
    nc = _get_module()
    in_maps = [
        {"x": x[c * ROWS_PER_CORE:(c + 1) * ROWS_PER_CORE]} for c in range(N_CORES)
    ]
    res = run_bass_kernel_spmd(
        nc, in_maps, core_ids=list(range(N_CORES)), trace=trace, **spmd_kwargs
    )
    vals = np.concatenate([r["out_vals"][:, :2] for r in res.results], axis=0)
    idx = np.concatenate([r["out_idx"][:, :2] for r in res.results], axis=0)
    topk_score = vals.reshape(NB, NC, NX, 2).astype(np.float32)
    topk_idx = idx.reshape(NB, NC, NX, 2).astype(np.int32)
    return (topk_score, topk_idx), res


def kernel(xcorr: np.ndarray, nlag=None, **_unused):
    out, _ = run(xcorr)
    return out
